# revision 11
# baseline (speedup 1.0000x reference)
"""ACAR head kernel for 8 Trainium2 NeuronCores.

Data-parallel over batch: each core processes 1 image (all 10 rois).
All convs are PE matmuls in float32r (TF32-like rounding, full rate at N>=256):
 - temporal mean folded into conv_reduce weights (x0.25)
 - roi_align lowered to a host-built sparse-as-dense [pix, bins] matrix
 - conv1 1x1 decomposed: relu(W_bg @ feats + W_actor @ actor_r) (bg shared across rois)
 - 3x3 convs: 9 shifted-AP accumulating matmuls (zero-padded SBUF tiles for SAME)
 - HR2O attention: q*k via DVE + PE one-hot column reduction; softmax on 10 partitions;
   att broadcast back to 128 partitions via PE; virt = sum_j att*v via DVE mult+reduce
 - GroupNorm stats via free-dim reduces + PE ones reduction; apply fused into ACT relu
Outputs per core: cls^T [80,10], roi_feats^T [1024,10]; host transposes/concats.
"""
import sys
from contextlib import ExitStack

import numpy as np

for _p in ('/opt/trn_rl_repo', '/root/.axon_site/_ro/trn_rl_repo'):
    if _p not in sys.path:
        sys.path.append(_p)

import concourse.bass as bass
import concourse.tile as tile
from concourse import mybir
from concourse.bass_utils import run_bass_kernel_spmd

F32 = mybir.dt.float32
F32R = mybir.dt.float32r

N_CORES = 8
B, R = 8, 10
CIN, T, H, W = 2048, 4, 16, 22
RED, HID, NCLS = 1024, 512, 80
ROI_SP, SR, DEPTH = 7, 2, 2
P = H * W                  # 352 pixels
NB = R * ROI_SP * ROI_SP   # 490 bins
H2, W2 = H - 2, W - 2      # 14, 20 conv2 VALID out
H3, W3 = 7, 10             # after maxpool 3x3 s2 p1
P3 = H3 * W3               # 70
HP, WP = H3 + 2, W3 + 2    # 9, 12 padded for SAME convs
PP = HP * WP               # 108
KC = CIN // 128            # 16
MR = RED // 128            # 8
MH = HID // 128            # 4
PIX_CH = [(0, 128), (128, 128), (256, 96)]
HF_CH = [(0, 40), (40, 30)]   # pixel splits at row boundaries (4 rows / 3 rows)
INV_SQRT_HID = 1.0 / float(np.sqrt(HID))


def _split_waits(nc):
    """This container's walrus rejects instructions with multiple sync waits.
    Hoist excess waits onto standalone wait-only InstEventSemaphore on the
    same engine (sequencer program order makes this equivalent)."""
    n = 0
    for f in nc.m.functions:
        for blk in f.blocks:
            if not any(ins.sync_info is not None and len(ins.sync_info.on_wait) > 1
                       for ins in blk.instructions):
                continue
            out = []
            for ins in blk.instructions:
                si = ins.sync_info
                if si is not None and len(si.on_wait) > 1:
                    waits = list(si.on_wait)
                    for j, w in enumerate(waits[:-1]):
                        out.append(mybir.InstEventSemaphore(
                            name=f"{ins.name}-ws{j}", engine=ins.engine,
                            sync_info=mybir.SyncInfo(on_wait=[w], on_update=[])))
                        n += 1
                    ins.sync_info = mybir.SyncInfo(on_wait=[waits[-1]],
                                                   on_update=list(si.on_update))
                out.append(ins)
            try:
                blk.instructions = out
            except Exception:
                blk.instructions.clear()
                blk.instructions.extend(out)
    return n


def _view(ap, offset_delta, dims):
    """Free-dim view of an AP: dims = [(step, count), ...]; keeps partitions."""
    return bass.AP(tensor=ap.tensor, offset=ap.offset + offset_delta,
                   ap=[list(ap.ap[0])] + [[s, c] for s, c in dims])


def build_nc(split=True, dbg=False):
    nc = bass.Bass("TRN2", target_bir_lowering=False, debug=False,
                   num_devices=N_CORES)

    def din(name, shape, dt=F32R):
        return nc.declare_dram_parameter(name, list(shape), dt, isOutput=False)

    x_d = din("x", [CIN, T * P], F32)
    wrT_d = din("wrT", [CIN, RED])
    spT_d = din("spT", [P, NB])
    w1bgT_d = din("w1bgT", [RED, HID])
    w1acT_d = din("w1acT", [RED, HID])
    w2T_d = din("w2T", [9, HID, HID])
    wqT_d = din("wqT", [DEPTH, 9, HID, HID])
    wkT_d = din("wkT", [DEPTH, 9, HID, HID])
    wvT_d = din("wvT", [DEPTH, 9, HID, HID])
    woT_d = din("woT", [DEPTH, 9, HID, HID])
    gng_d = din("gng", [DEPTH, HID], F32)
    gnb_d = din("gnb", [DEPTH, HID], F32)
    wfc1T_d = din("wfc1T", [RED, HID])
    wfc2T_d = din("wfc2T", [2 * HID, NCLS])
    ident_d = din("ident", [128, 128])
    ones_d = din("ones", [128, 1])
    onesr_d = din("onesr", [1, 128])
    e_d = din("erep", [R, R * 128])
    e3_d = din("e3", [128, R * R])

    dbgd = {}
    if dbg:
        for nm, shp in [("dbg_bg", [MH, 128, P]), ("dbg_ac", [MH, 128, R]),
                        ("dbg_int", [MH, 128, P]),
                        ("dbg_xhr0", [MH, 128, R * HP * WP]),
                        ("dbg_q", [MH, 128, R * P3]), ("dbg_k", [MH, 128, R * P3]),
                        ("dbg_v", [MH, 128, R * P3]), ("dbg_att", [R, P3 * R]),
                        ("dbg_vpraw", [MH, 128, R * HP * WP]),
                        ("dbg_vpact", [MH, 128, R * HP * WP]),
                        ("dbg_xhr1", [MH, 128, R * HP * WP]),
                        ("dbg_hof", [MH, 128, R]), ("dbg_ofc", [MH, 128, R])]:
            dbgd[nm] = nc.declare_dram_parameter(nm, shp, F32, isOutput=True)

    cls_d = nc.declare_dram_parameter("cls", [NCLS, R], F32R, isOutput=True)
    rf_d = nc.declare_dram_parameter("rf", [RED, R], F32R, isOutput=True)

    dma = nc.sync.dma_start
    AF = mybir.ActivationFunctionType
    AL = mybir.AluOpType
    AX = mybir.AxisListType

    with nc.allow_low_precision(reason="f32r tiles are fp32-width"), \
         tile.TileContext(nc) as tc, ExitStack() as es:
        const = es.enter_context(tc.tile_pool(name="const", bufs=1))
        stream = es.enter_context(tc.tile_pool(name="stream", bufs=1))
        scr = es.enter_context(tc.tile_pool(name="scr", bufs=2))
        fcw = es.enter_context(tc.tile_pool(name="fcw", bufs=1))

        # ---- constants
        ident = const.tile([128, 128], F32R, name="ident")
        dma(out=ident, in_=ident_d[:, :])
        ones = const.tile([128, 1], F32R, name="ones")
        dma(out=ones, in_=ones_d[:, :])
        onesr = const.tile([1, 128], F32R, name="onesr")
        dma(out=onesr, in_=onesr_d[:, :])
        erep = const.tile([R, R * 128], F32R, name="erep")
        dma(out=erep, in_=e_d[:, :])
        e3 = const.tile([128, R * R], F32R, name="e3")
        dma(out=e3, in_=e3_d[:, :])
        gng = const.tile([128, DEPTH * MH], F32, name="gng")
        gnb = const.tile([128, DEPTH * MH], F32, name="gnb")
        for d in range(DEPTH):
            for m in range(MH):
                col = d * MH + m
                dma(out=gng[:, col:col + 1],
                    in_=gng_d[d, m * 128:(m + 1) * 128].rearrange("(p o) -> p o", o=1))
                dma(out=gnb[:, col:col + 1],
                    in_=gnb_d[d, m * 128:(m + 1) * 128].rearrange("(p o) -> p o", o=1))
        eps = const.tile([1, 1], F32, name="eps")
        nc.vector.memset(eps, 1e-5)
        neg1 = const.tile([128, 1], F32, name="neg1")
        nc.vector.memset(neg1, -1.0)

        wfc1 = [fcw.tile([128, HID], F32R, name=f"wfc1_{k}") for k in range(MR)]
        for k in range(MR):
            dma(out=wfc1[k], in_=wfc1T_d[k * 128:(k + 1) * 128, :])
        wfc2 = [fcw.tile([128, NCLS], F32R, name=f"wfc2_{k}") for k in range(MR)]
        for k in range(MR):
            dma(out=wfc2[k], in_=wfc2T_d[k * 128:(k + 1) * 128, :])
        roi_cT = [fcw.tile([128, R], F32R, name=f"roi{m}") for m in range(MR)]

        # x_hr and vp live from P5 to the end; open early for LIFO stacking
        hrx = es.enter_context(tc.tile_pool(name="hrx", bufs=1))
        x_hr = [hrx.tile([128, R, HP, WP], F32R, name=f"xhr{m}") for m in range(MH)]
        vp = [hrx.tile([128, R, HP, WP], F32R, name=f"vp{m}") for m in range(MH)]
        for m in range(MH):
            nc.vector.memset(x_hr[m][:, :, :, :].bitcast(F32), 0.0)
            nc.vector.memset(vp[m][:, :, :, :].bitcast(F32), 0.0)

        # ============ P1: temporal mean + conv_reduce -> featsT [pix, c]
        with tc.tile_pool(name="ftp", bufs=1) as ftp:
            featsT = [ftp.tile([128, RED], F32R, name=f"fT{m}") for m in range(3)]
            spT = [ftp.tile([128, NB], F32R, name=f"sp{m}") for m in range(3)]
            for m, (off, cnt) in enumerate(PIX_CH):
                dma(out=spT[m][:cnt, :], in_=spT_d[off:off + cnt, :])

            with tc.tile_pool(name="fcp", bufs=1) as fcp, \
                 tc.tile_pool(name="psA", bufs=1, space="PSUM") as psA:
                feats_cp = [fcp.tile([128, P], F32R, name=f"fcp{k}")
                            for k in range(KC)]
                psT = [psA.tile([128, RED], F32, name=f"psT{m}", tag=f"psT{m}")
                       for m in range(3)]
                for k in range(KC):
                    xt = fcp.tile([128, T * P], F32, name="xt", tag="xt", bufs=3)
                    dma(out=xt, in_=x_d[k * 128:(k + 1) * 128, :])
                    xv = _view(xt, 0, [(1, P), (P, T)])  # sum over t innermost
                    nc.vector.tensor_reduce(out=feats_cp[k], in_=xv,
                                            axis=AX.X, op=AL.add)
                    wr = stream.tile([128, RED], F32R, name="wr", tag="wr", bufs=3)
                    dma(out=wr, in_=wrT_d[k * 128:(k + 1) * 128, :])
                    for m, (off, cnt) in enumerate(PIX_CH):
                        for hh in range(2):
                            nc.tensor.matmul(
                                psT[m][:cnt, hh * 512:(hh + 1) * 512],
                                feats_cp[k][:, off:off + cnt],
                                wr[:, hh * 512:(hh + 1) * 512],
                                start=(k == 0), stop=(k == KC - 1))
                for m, (off, cnt) in enumerate(PIX_CH):
                    nc.scalar.copy(featsT[m][:cnt, :], psT[m][:cnt, :])

            # ============ P2+P3: transpose -> feats_red; roi bins -> roi_cT
            with tc.tile_pool(name="frp", bufs=1) as frp:
                feats_red = [frp.tile([128, P], F32R, name=f"fr{m}")
                             for m in range(MR)]
                with tc.tile_pool(name="psB", bufs=2, space="PSUM") as psB:
                    for m in range(MR):
                        for pc, (off, cnt) in enumerate(PIX_CH):
                            tp = psB.tile([128, 128], F32R, name="ptr", tag="ptr")
                            nc.tensor.transpose(
                                tp[:, :cnt], featsT[pc][:cnt, m * 128:(m + 1) * 128],
                                ident[:cnt, :cnt])
                            nc.scalar.copy(feats_red[m][:, off:off + cnt], tp[:, :cnt])
                    for m in range(MR):
                        pb = psB.tile([128, NB], F32, name="pbin", tag="pbin")
                        for pc, (off, cnt) in enumerate(PIX_CH):
                            nc.tensor.matmul(pb,
                                             featsT[pc][:cnt, m * 128:(m + 1) * 128],
                                             spT[pc][:cnt, :],
                                             start=(pc == 0), stop=(pc == 2))
                        pbv = _view(pb, 0, [(49, R), (1, 49)])
                        nc.vector.tensor_reduce(out=roi_cT[m], in_=pbv,
                                                axis=AX.X, op=AL.max)

                # ============ P4: conv1 bg (shared) + actor parts
                with tc.tile_pool(name="c2p", bufs=1) as c2p:
                    bg_sb = [c2p.tile([128, P], F32, name=f"bg{m}")
                             for m in range(MH)]
                    actor = [c2p.tile([128, R], F32, name=f"acr{m}")
                             for m in range(MH)]
                    with tc.tile_pool(name="psC", bufs=1, space="PSUM") as psC:
                        pbg = [psC.tile([128, P], F32, name=f"pbg{m}", tag=f"pbg{m}")
                               for m in range(MH)]
                        pac = [psC.tile([128, R], F32, name=f"pac{m}", tag=f"pac{m}")
                               for m in range(MH)]
                        for k in range(MR):
                            wb = stream.tile([128, HID], F32R, name="w1",
                                             tag="w1", bufs=3)
                            dma(out=wb, in_=w1bgT_d[k * 128:(k + 1) * 128, :])
                            for m in range(MH):
                                nc.tensor.matmul(pbg[m], wb[:, m * 128:(m + 1) * 128],
                                                 feats_red[k],
                                                 start=(k == 0), stop=(k == MR - 1))
                        for k in range(MR):
                            wa = stream.tile([128, HID], F32R, name="w1a",
                                             tag="w1", bufs=3)
                            dma(out=wa, in_=w1acT_d[k * 128:(k + 1) * 128, :])
                            for m in range(MH):
                                nc.tensor.matmul(pac[m], wa[:, m * 128:(m + 1) * 128],
                                                 roi_cT[k],
                                                 start=(k == 0), stop=(k == MR - 1))
                        for m in range(MH):
                            nc.scalar.copy(bg_sb[m], pbg[m])
                            nc.scalar.copy(actor[m], pac[m])
                        if dbg:
                            for m in range(MH):
                                dma(out=dbgd["dbg_bg"][m, :, :], in_=bg_sb[m])
                                dma(out=dbgd["dbg_ac"][m, :, :], in_=actor[m])

                    # ======== P5: conv2 (VALID) + relu + maxpool -> x_hr
                    mp = [c2p.tile([128, H, W], F32, name=f"mp{m}")
                          for m in range(MH)]
                    for m in range(MH):
                        nc.vector.memset(mp[m][:, :, :], 0.0)
                    with tc.tile_pool(name="w2p", bufs=1) as w2p, \
                         tc.tile_pool(name="psD", bufs=2, space="PSUM") as psD:
                        for oh in range(2):
                            # per cin-chunk tiles holding this oc-half's weights
                            w2 = [w2p.tile([128, 9, HID // 2], F32R, name=f"w2_{k}",
                                           tag=f"w2_{k}") for k in range(MH)]
                            for k in range(MH):
                                dma(out=w2[k],
                                    in_=w2T_d[:, k * 128:(k + 1) * 128,
                                              oh * 256:(oh + 1) * 256]
                                    .rearrange("s p c -> p s c"))
                            for r in range(R):
                                inter = [c2p.tile([128, P], F32R, name=f"int{m}",
                                                  tag=f"int{m}", bufs=2)
                                         for m in range(MH)]
                                for m in range(MH):
                                    nc.scalar.activation(
                                        out=inter[m], in_=bg_sb[m], func=AF.Relu,
                                        bias=actor[m][:, r:r + 1], scale=1.0)
                                if dbg and oh == 0 and r == 0:
                                    for m in range(MH):
                                        dma(out=dbgd["dbg_int"][m, :, :],
                                            in_=inter[m][:, :].bitcast(F32))
                                for j in range(2):
                                    oc = oh * 2 + j
                                    pc2 = psD.tile([128, H2 * W2], F32,
                                                   name="pc2", tag="pc2")
                                    first = True
                                    for k in range(MH):
                                        for s in range(9):
                                            dy, dx = s // 3, s % 3
                                            rhs = _view(inter[k], dy * W + dx,
                                                        [(W, H2), (1, W2)])
                                            nc.tensor.matmul(
                                                pc2,
                                                w2[k][:, s, j * 128:(j + 1) * 128],
                                                rhs, start=first,
                                                stop=(k == MH - 1 and s == 8))
                                            first = False
                                    nc.scalar.activation(
                                        out=mp[oc][:, 1:1 + H2, 1:1 + W2],
                                        in_=pc2.rearrange("p (a b) -> p a b", b=W2),
                                        func=AF.Relu)
                                    win = _view(mp[oc], 0,
                                                [(2 * W, H3), (2, W3), (W, 3), (1, 3)])
                                    nc.vector.tensor_reduce(
                                        out=x_hr[oc][:, r, 1:1 + H3, 1:1 + W3],
                                        in_=win, axis=AX.XY, op=AL.max)

        # ============ P6: HR2O x DEPTH
        if dbg:
            for m in range(MH):
                dma(out=dbgd["dbg_xhr0"][m, :, :],
                    in_=x_hr[m].rearrange("p a b c -> p (a b c)").bitcast(F32))
        hrp = es.enter_context(tc.tile_pool(name="hrp", bufs=1))
        qsb = [hrp.tile([128, R, P3], F32, name=f"q{m}") for m in range(MH)]
        ksb = [hrp.tile([128, R, P3], F32, name=f"k{m}") for m in range(MH)]
        vsb = [hrp.tile([128, R, P3], F32, name=f"v{m}") for m in range(MH)]
        att_sm = hrp.tile([R, P3, R], F32R, name="attsm")
        st = [hrp.tile([128, 2 * R], F32R, name=f"st{m}") for m in range(MH)]
        mrb = hrp.tile([128, 2 * R], F32, name="mrb")
        al = [hrp.tile([128, R], F32, name=f"al{m}") for m in range(MH)]
        be = [hrp.tile([128, R], F32, name=f"be{m}") for m in range(MH)]

        def conv3x3_same(w_dram, src_tiles, sink):
            """3x3 SAME conv on [HID, 10, (7,10)] in padded (9,12) tiles.
            (kc, s)-outer streams each weight tile once; 8 psum accumulators."""
            with tc.tile_pool(name="pcv", bufs=1, space="PSUM") as pcvp:
                pcv = [pcvp.tile([128, 5 * P3], F32, name=f"pcv{i}", tag=f"pcv{i}")
                       for i in range(8)]
                for k in range(MH):
                    for s in range(9):
                        dy, dx = s // 3, s % 3
                        wt = stream.tile([128, HID], F32R, name="wc",
                                         tag="wc", bufs=8)
                        dma(out=wt, in_=w_dram[s, k * 128:(k + 1) * 128, :])
                        for oc in range(MH):
                            for g in range(2):
                                rhs = _view(src_tiles[k],
                                            g * 5 * PP + dy * WP + dx,
                                            [(PP, 5), (WP, H3), (1, W3)])
                                nc.tensor.matmul(
                                    pcv[oc * 2 + g],
                                    wt[:, oc * 128:(oc + 1) * 128], rhs,
                                    start=(k == 0 and s == 0),
                                    stop=(k == MH - 1 and s == 8))
                for oc in range(MH):
                    for g in range(2):
                        sink(oc, g, pcv[oc * 2 + g])

        def mk_copy(dst, scale=None):
            def _sink(oc, g, ps):
                o = dst[oc][:, g * 5:(g + 1) * 5, :]
                src = ps.rearrange("p (a b) -> p a b", b=P3)
                if scale is None:
                    nc.scalar.copy(o, src)
                else:
                    nc.scalar.mul(o, src, scale)
            return _sink

        for d in range(DEPTH):
            conv3x3_same(wqT_d[d], x_hr, mk_copy(qsb, INV_SQRT_HID))
            conv3x3_same(wkT_d[d], x_hr, mk_copy(ksb))
            conv3x3_same(wvT_d[d], x_hr, mk_copy(vsb))
            if dbg and d == 0:
                for m in range(MH):
                    dma(out=dbgd["dbg_q"][m, :, :], in_=qsb[m].rearrange("p a b -> p (a b)"))
                    dma(out=dbgd["dbg_k"][m, :, :], in_=ksb[m].rearrange("p a b -> p (a b)"))
                    dma(out=dbgd["dbg_v"][m, :, :], in_=vsb[m].rearrange("p a b -> p (a b)"))

            with tc.tile_pool(name="psE", bufs=2, space="PSUM") as psE:
                # att[i, j, p] = sum_c q[c,i,p] k[c,j,p], softmax over j
                for hf, (po, pn) in enumerate(HF_CH):
                    patt = psE.tile([R, 40 * R], F32, name="patt", tag="patt")
                    first = True
                    for i in range(R):
                        for c in range(MH):
                            q_b = qsb[c][:, i, po:po + pn] \
                                .broadcast_to([128, pn, R])
                            k_b = _view(ksb[c], po, [(1, pn), (P3, R)])
                            prod = scr.tile([128, 40 * R], F32R, name="prod",
                                            tag="prod", bufs=4)
                            nc.vector.tensor_tensor(
                                out=_view(prod, 0, [(R, pn), (1, R)]),
                                in0=q_b, in1=k_b, op=AL.mult)
                            nc.tensor.matmul(
                                patt[:, :pn * R], e3[:, i * R:(i + 1) * R],
                                prod[:, :pn * R], start=first,
                                stop=(i == R - 1 and c == MH - 1))
                            first = False
                    pattv = _view(patt, 0, [(R, pn), (1, R)])
                    mx = scr.tile([R, 40], F32, name="mx", tag="mx")
                    nc.vector.tensor_reduce(out=mx[:, :pn], in_=pattv,
                                            axis=AX.X, op=AL.max)
                    att_c = scr.tile([R, 40 * R], F32, name="attc", tag="attc")
                    nc.vector.tensor_tensor(
                        out=_view(att_c, 0, [(R, pn), (1, R)]),
                        in0=pattv, in1=mx[:, :pn].broadcast_to([R, pn, R]),
                        op=AL.subtract)
                    att_e = scr.tile([R, 40 * R], F32, name="atte", tag="atte")
                    nc.scalar.activation(out=att_e[:, :pn * R],
                                         in_=att_c[:, :pn * R], func=AF.Exp)
                    sm = scr.tile([R, 40], F32, name="sm", tag="sm")
                    nc.vector.tensor_reduce(
                        out=sm[:, :pn], in_=_view(att_e, 0, [(R, pn), (1, R)]),
                        axis=AX.X, op=AL.add)
                    rs = scr.tile([R, 40], F32, name="rs", tag="rs")
                    nc.vector.reciprocal(out=rs[:, :pn], in_=sm[:, :pn])
                    nc.vector.tensor_tensor(
                        out=att_sm[:, po:po + pn, :],
                        in0=_view(att_e, 0, [(R, pn), (1, R)]),
                        in1=rs[:, :pn].broadcast_to([R, pn, R]), op=AL.mult)

                # attb (replicated) + virt accumulated straight into vp interior
                for hf, (po, pn) in enumerate(HF_CH):
                    attb = hrp.tile([128, R, 40, R], F32, name="attb",
                                    tag="attb", bufs=2)
                    for i in range(R):
                        prep = psE.tile([128, 40 * R], F32, name="prep",
                                        tag="prep")
                        nc.tensor.matmul(prep[:, :pn * R],
                                         erep[:, i * 128:(i + 1) * 128],
                                         att_sm[:, po:po + pn, :],
                                         start=True, stop=True)
                        nc.scalar.copy(attb[:, i, :pn, :],
                                       _view(prep, 0, [(R, pn), (1, R)]))
                    yo, yn = (1, 4) if hf == 0 else (5, 3)
                    for c in range(MH):
                        for i in range(R):
                            v_pj = _view(vsb[c], po, [(1, pn), (P3, R)])
                            p2 = scr.tile([128, 40 * R], F32, name="p2",
                                          tag="p2")
                            nc.vector.tensor_tensor(
                                out=_view(p2, 0, [(R, pn), (1, R)]),
                                in0=v_pj, in1=attb[:, i, :pn, :], op=AL.mult)
                            nc.vector.tensor_reduce(
                                out=vp[c][:, i, yo:yo + yn, 1:1 + W3],
                                in_=_view(p2, 0, [(10 * R, yn), (R, W3), (1, R)]),
                                axis=AX.X, op=AL.add)

                if dbg and d == 0:
                    dma(out=dbgd["dbg_att"][:, :],
                        in_=att_sm.rearrange("p a b -> p (a b)").bitcast(F32))
                    for m in range(MH):
                        dma(out=dbgd["dbg_vpraw"][m, :, :],
                            in_=vp[m].rearrange("p a b c -> p (a b c)").bitcast(F32))

                # GroupNorm(1, HID) per actor over (c, p): stats from vp interior
                for c in range(MH):
                    vin = vp[c][:, :, 1:1 + H3, 1:1 + W3]
                    nc.vector.tensor_reduce(out=st[c][:, 0:R], in_=vin,
                                            axis=AX.XY, op=AL.add)
                    sq = scr.tile([128, R * P3], F32, name="sq", tag="sq")
                    nc.vector.tensor_tensor(
                        out=sq.rearrange("p (a b c) -> p a b c", b=H3, c=W3),
                        in0=vin, in1=vin, op=AL.mult)
                    nc.vector.tensor_reduce(
                        out=st[c][:, R:2 * R],
                        in_=sq.rearrange("p (a b) -> p a b", b=P3),
                        axis=AX.X, op=AL.add)
                pgn = psE.tile([1, 2 * R], F32, name="pgn", tag="pgn")
                for c in range(MH):
                    nc.tensor.matmul(pgn, ones, st[c], start=(c == 0),
                                     stop=(c == MH - 1))
                mr = scr.tile([1, 2 * R], F32R, name="mr", tag="mr")
                ninv = 1.0 / float(HID * P3)
                nc.vector.tensor_scalar(out=mr, in0=pgn, scalar1=ninv,
                                        scalar2=None, op0=AL.mult)
                musq = scr.tile([1, R], F32, name="musq", tag="musq")
                nc.vector.tensor_tensor(out=musq, in0=mr[:, 0:R].bitcast(F32),
                                        in1=mr[:, 0:R].bitcast(F32), op=AL.mult)
                var = scr.tile([1, R], F32, name="var", tag="var")
                nc.vector.tensor_tensor(out=var, in0=mr[:, R:2 * R].bitcast(F32),
                                        in1=musq, op=AL.subtract)
                sd = scr.tile([1, R], F32, name="sd", tag="sd")
                nc.scalar.activation(out=sd, in_=var, func=AF.Sqrt,
                                     bias=eps[:1, :], scale=1.0)
                nc.vector.reciprocal(out=mr[:, R:2 * R], in_=sd)
                pgr = psE.tile([128, 2 * R], F32, name="pgr", tag="pgr")
                nc.tensor.matmul(pgr, onesr, mr, start=True, stop=True)
                nc.scalar.copy(mrb, pgr)

            for c in range(MH):
                gi = d * MH + c
                nc.vector.tensor_scalar_mul(out=al[c], in0=mrb[:, R:2 * R],
                                            scalar1=gng[:, gi:gi + 1])
                ma = scr.tile([128, R], F32, name="ma", tag="ma")
                nc.vector.tensor_tensor(out=ma, in0=mrb[:, 0:R], in1=al[c],
                                        op=AL.mult)
                nc.vector.tensor_scalar_mul(out=ma, in0=ma, scalar1=neg1)
                nc.vector.tensor_scalar_add(out=be[c], in0=ma,
                                            scalar1=gnb[:, gi:gi + 1])
                for i in range(R):
                    vint = vp[c][:, i, 1:1 + H3, 1:1 + W3]
                    nc.scalar.activation(out=vint, in_=vint, func=AF.Relu,
                                         scale=al[c][:, i:i + 1],
                                         bias=be[c][:, i:i + 1])

            if dbg and d == 0:
                for m in range(MH):
                    dma(out=dbgd["dbg_vpact"][m, :, :],
                        in_=vp[m].rearrange("p a b c -> p (a b c)").bitcast(F32))

            # o-conv + residual add into x_hr (in place)
            def mk_res(oc, g, ps):
                tgt = x_hr[oc][:, g * 5:(g + 1) * 5, 1:1 + H3, 1:1 + W3]
                nc.vector.tensor_tensor(
                    out=tgt,
                    in0=ps.rearrange("p (a b c) -> p a b c", b=H3, c=W3),
                    in1=tgt, op=AL.add)
            conv3x3_same(woT_d[d], vp, mk_res)
            if dbg and d == 0:
                for m in range(MH):
                    dma(out=dbgd["dbg_xhr1"][m, :, :],
                        in_=x_hr[m].rearrange("p a b c -> p (a b c)").bitcast(F32))

        # ============ P7: hof, fc1, fc2, outputs
        with tc.tile_pool(name="psF", bufs=1, space="PSUM") as psF:
            hof = [hrp.tile([128, R], F32R, name=f"hof{m}") for m in range(MH)]
            for c in range(MH):
                nc.vector.tensor_reduce(
                    out=hof[c], in_=x_hr[c][:, :, 1:1 + H3, 1:1 + W3],
                    axis=AX.XY, op=AL.add)
            ofc1 = [hrp.tile([128, R], F32R, name=f"ofc{m}") for m in range(MH)]
            pfc = [psF.tile([128, R], F32, name=f"pfc{m}", tag=f"pfc{m}")
                   for m in range(MH)]
            for k in range(MR):
                for m in range(MH):
                    nc.tensor.matmul(pfc[m], wfc1[k][:, m * 128:(m + 1) * 128],
                                     roi_cT[k], start=(k == 0), stop=(k == MR - 1))
            for m in range(MH):
                nc.scalar.activation(out=ofc1[m], in_=pfc[m], func=AF.Relu)
            if dbg:
                for m in range(MH):
                    dma(out=dbgd["dbg_hof"][m, :, :], in_=hof[m][:, :].bitcast(F32))
                    dma(out=dbgd["dbg_ofc"][m, :, :], in_=ofc1[m][:, :].bitcast(F32))
            pcls = psF.tile([NCLS, R], F32, name="pcls", tag="pcls")
            for k in range(MR):
                rhs = ofc1[k] if k < MH else hof[k - MH]
                nc.tensor.matmul(pcls, wfc2[k], rhs, start=(k == 0),
                                 stop=(k == MR - 1))
            cls_sb = fcw.tile([NCLS, R], F32R, name="cls_sb")
            nc.scalar.copy(cls_sb, pcls)
            dma(out=cls_d[:, :], in_=cls_sb)
            for m in range(MR):
                dma(out=rf_d[m * 128:(m + 1) * 128, :], in_=roi_cT[m])

    if split:
        _split_waits(nc)
    return nc


# ---------------------------------------------------------------- host side
def _roi_matrix(rois_px):
    """rois_px [R,5] pixel coords -> S^T [P, R*49] f32 (roi_align + bin mean),
    float32 coordinate arithmetic matching the jax reference."""
    S = np.zeros((NB, P), np.float64)
    grid = (np.arange(ROI_SP, dtype=np.float32)[:, None]
            + (np.arange(SR, dtype=np.float32)[None, :] + 0.5)
            / np.float32(SR)).reshape(-1)
    for r in range(R):
        x1, y1, x2, y2 = [np.float32(v) for v in rois_px[r, 1:5]]
        bw = np.maximum(x2 - x1, np.float32(1.0)) / np.float32(ROI_SP)
        bh = np.maximum(y2 - y1, np.float32(1.0)) / np.float32(ROI_SP)
        ys = (y1 + grid * bh).astype(np.float32)
        xs = (x1 + grid * bw).astype(np.float32)
        ym = (ys >= -1.0) & (ys <= H)
        xm = (xs >= -1.0) & (xs <= W)
        yv = np.clip(ys, np.float32(0.0), np.float32(H - 1))
        xv = np.clip(xs, np.float32(0.0), np.float32(W - 1))
        y0 = np.floor(yv).astype(np.int64)
        x0 = np.floor(xv).astype(np.int64)
        yh = np.minimum(y0 + 1, H - 1)
        xh = np.minimum(x0 + 1, W - 1)
        ly = (yv - y0).astype(np.float64)
        lx = (xv - x0).astype(np.float64)
        wgt = 1.0 / (SR * SR)
        for iy in range(ROI_SP * SR):
            if not ym[iy]:
                continue
            for ix in range(ROI_SP * SR):
                if not xm[ix]:
                    continue
                b = r * 49 + (iy // SR) * ROI_SP + (ix // SR)
                S[b, y0[iy] * W + x0[ix]] += wgt * (1 - ly[iy]) * (1 - lx[ix])
                S[b, y0[iy] * W + xh[ix]] += wgt * (1 - ly[iy]) * lx[ix]
                S[b, yh[iy] * W + x0[ix]] += wgt * ly[iy] * (1 - lx[ix])
                S[b, yh[iy] * W + xh[ix]] += wgt * ly[iy] * lx[ix]
    return np.ascontiguousarray(S.T.astype(np.float32))


_NC_CACHE = {}


def _get_nc():
    if "nc" not in _NC_CACHE:
        _NC_CACHE["nc"] = build_nc()
    return _NC_CACHE["nc"]


def _conv_T(w):  # [O, I, 3, 3] -> [9, I, O]
    return np.ascontiguousarray(
        w.transpose(2, 3, 1, 0).reshape(9, w.shape[1], w.shape[0]))


def kernel(x, rois, w_reduce, w_conv1, w_conv2, w_q, w_k, w_v, w_o,
           gn_g, gn_b, w_fc1, w_fc2):
    f32 = np.float32
    x = np.asarray(x, f32)
    rois = np.asarray(rois, f32)
    nc = _get_nc()

    wrT = np.ascontiguousarray((np.asarray(w_reduce, f32) * (1.0 / T)).T)
    w1 = np.asarray(w_conv1, f32)
    w1bgT = np.ascontiguousarray(w1[:, :RED].T)
    w1acT = np.ascontiguousarray(w1[:, RED:].T)
    w2T = _conv_T(np.asarray(w_conv2, f32))
    wqT = np.stack([_conv_T(np.asarray(w_q, f32)[d]) for d in range(DEPTH)])
    wkT = np.stack([_conv_T(np.asarray(w_k, f32)[d]) for d in range(DEPTH)])
    wvT = np.stack([_conv_T(np.asarray(w_v, f32)[d]) for d in range(DEPTH)])
    woT = np.stack([_conv_T(np.asarray(w_o, f32)[d]) for d in range(DEPTH)])
    wfc1T = np.ascontiguousarray(np.asarray(w_fc1, f32).T)
    wfc2T = np.ascontiguousarray(np.asarray(w_fc2, f32).T)
    wfc2T[HID:, :] *= 1.0 / P3  # fold hof spatial mean
    ident = np.eye(128, dtype=f32)
    ones = np.ones((128, 1), f32)
    onesr = np.ones((1, 128), f32)
    erep = np.zeros((R, R * 128), f32)
    for i in range(R):
        erep[i, i * 128:(i + 1) * 128] = 1.0
    e3 = np.zeros((128, R * R), f32)
    for i in range(R):
        e3[:, i * R + i] = 1.0

    scale = np.array([1.0, W, H, W, H], f32)
    in_maps = []
    for c in range(N_CORES):
        rois_px = rois[c * R:(c + 1) * R] * scale[None, :]
        in_maps.append({
            "x": np.ascontiguousarray(x[c].reshape(CIN, T * P)),
            "wrT": wrT, "spT": _roi_matrix(rois_px),
            "w1bgT": w1bgT, "w1acT": w1acT, "w2T": w2T,
            "wqT": wqT, "wkT": wkT, "wvT": wvT, "woT": woT,
            "gng": np.asarray(gn_g, f32), "gnb": np.asarray(gn_b, f32),
            "wfc1T": wfc1T, "wfc2T": wfc2T,
            "ident": ident, "ones": ones, "onesr": onesr,
            "erep": erep, "e3": e3,
        })

    res = run_bass_kernel_spmd(nc, in_maps, core_ids=list(range(N_CORES)),
                               trace=False)
    _NC_CACHE["last_res"] = res
    cls = np.empty((B * R, NCLS), f32)
    rf = np.empty((B * R, RED), f32)
    for c in range(N_CORES):
        cls[c * R:(c + 1) * R] = res.results[c]["cls"].T
        rf[c * R:(c + 1) * R] = res.results[c]["rf"].T
    return cls, rf


# revision 12
# speedup vs baseline: 1.1063x; 1.1063x over previous
"""ACAR head kernel for 8 Trainium2 NeuronCores.

Data-parallel over batch: each core processes 1 image (all 10 rois).
All convs are PE matmuls in float32r (TF32-like rounding, full rate at N>=256):
 - temporal mean folded into conv_reduce weights (x0.25)
 - roi_align lowered to a host-built sparse-as-dense [pix, bins] matrix
 - conv1 1x1 decomposed: relu(W_bg @ feats + W_actor @ actor_r) (bg shared across rois)
 - 3x3 convs: 9 shifted-AP accumulating matmuls (zero-padded SBUF tiles for SAME)
 - HR2O attention: q*k via DVE + PE one-hot column reduction; softmax on 10 partitions;
   att broadcast back to 128 partitions via PE; virt = sum_j att*v via DVE mult+reduce
 - GroupNorm stats via free-dim reduces + PE ones reduction; apply fused into ACT relu
Outputs per core: cls^T [80,10], roi_feats^T [1024,10]; host transposes/concats.
"""
import sys
from contextlib import ExitStack

import numpy as np
import ml_dtypes

for _p in ('/opt/trn_rl_repo', '/root/.axon_site/_ro/trn_rl_repo'):
    if _p not in sys.path:
        sys.path.append(_p)

import concourse.bass as bass
import concourse.tile as tile
from concourse import mybir
from concourse.bass_utils import run_bass_kernel_spmd

F32 = mybir.dt.float32
F32R = mybir.dt.float32r
BF16 = mybir.dt.bfloat16

N_CORES = 8
B, R = 8, 10
CIN, T, H, W = 2048, 4, 16, 22
RED, HID, NCLS = 1024, 512, 80
ROI_SP, SR, DEPTH = 7, 2, 2
P = H * W                  # 352 pixels
NB = R * ROI_SP * ROI_SP   # 490 bins
H2, W2 = H - 2, W - 2      # 14, 20 conv2 VALID out
H3, W3 = 7, 10             # after maxpool 3x3 s2 p1
P3 = H3 * W3               # 70
HP, WP = H3 + 2, W3 + 2    # 9, 12 padded for SAME convs
PP = HP * WP               # 108
KC = CIN // 128            # 16
MR = RED // 128            # 8
MH = HID // 128            # 4
PIX_CH = [(0, 128), (128, 128), (256, 96)]
HF_CH = [(0, 40), (40, 30)]   # pixel splits at row boundaries (4 rows / 3 rows)
INV_SQRT_HID = 1.0 / float(np.sqrt(HID))


def _split_waits(nc):
    """This container's walrus rejects instructions with multiple sync waits.
    Hoist excess waits onto standalone wait-only InstEventSemaphore on the
    same engine (sequencer program order makes this equivalent)."""
    n = 0
    for f in nc.m.functions:
        for blk in f.blocks:
            if not any(ins.sync_info is not None and len(ins.sync_info.on_wait) > 1
                       for ins in blk.instructions):
                continue
            out = []
            for ins in blk.instructions:
                si = ins.sync_info
                if si is not None and len(si.on_wait) > 1:
                    waits = list(si.on_wait)
                    for j, w in enumerate(waits[:-1]):
                        out.append(mybir.InstEventSemaphore(
                            name=f"{ins.name}-ws{j}", engine=ins.engine,
                            sync_info=mybir.SyncInfo(on_wait=[w], on_update=[])))
                        n += 1
                    ins.sync_info = mybir.SyncInfo(on_wait=[waits[-1]],
                                                   on_update=list(si.on_update))
                out.append(ins)
            try:
                blk.instructions = out
            except Exception:
                blk.instructions.clear()
                blk.instructions.extend(out)
    return n


def _view(ap, offset_delta, dims):
    """Free-dim view of an AP: dims = [(step, count), ...]; keeps partitions."""
    return bass.AP(tensor=ap.tensor, offset=ap.offset + offset_delta,
                   ap=[list(ap.ap[0])] + [[s, c] for s, c in dims])


def build_nc(split=True, dbg=False):
    nc = bass.Bass("TRN2", target_bir_lowering=False, debug=False,
                   num_devices=N_CORES)

    def din(name, shape, dt=F32R):
        return nc.declare_dram_parameter(name, list(shape), dt, isOutput=False)

    x_d = din("x", [CIN, T * P], F32)
    wrT_d = din("wrT", [CIN, RED])
    spT_d = din("spT", [P, NB])
    w1bgT_d = din("w1bgT", [RED, HID])
    w1acT_d = din("w1acT", [RED, HID])
    w2T_d = din("w2T", [9, HID, HID], BF16)
    wqT_d = din("wqT", [DEPTH, 9, HID, HID], BF16)
    wkT_d = din("wkT", [DEPTH, 9, HID, HID], BF16)
    wvT_d = din("wvT", [DEPTH, 9, HID, HID], BF16)
    woT_d = din("woT", [DEPTH, 9, HID, HID], BF16)
    gng_d = din("gng", [DEPTH, HID], F32)
    gnb_d = din("gnb", [DEPTH, HID], F32)
    wfc1T_d = din("wfc1T", [RED, HID])
    wfc2T_d = din("wfc2T", [2 * HID, NCLS])
    ident_d = din("ident", [128, 128])
    ones_d = din("ones", [128, 1])
    onesr_d = din("onesr", [1, 128])
    e_d = din("erep", [R, R * 128])
    e3_d = din("e3", [128, R * R])

    dbgd = {}
    if dbg:
        for nm, shp in [("dbg_bg", [MH, 128, P]), ("dbg_ac", [MH, 128, R]),
                        ("dbg_int", [MH, 128, P]),
                        ("dbg_xhr0", [MH, 128, R * HP * WP]),
                        ("dbg_q", [MH, 128, R * P3]), ("dbg_k", [MH, 128, R * P3]),
                        ("dbg_v", [MH, 128, R * P3]), ("dbg_att", [R, P3 * R]),
                        ("dbg_vpraw", [MH, 128, R * HP * WP]),
                        ("dbg_vpact", [MH, 128, R * HP * WP]),
                        ("dbg_xhr1", [MH, 128, R * HP * WP]),
                        ("dbg_hof", [MH, 128, R]), ("dbg_ofc", [MH, 128, R])]:
            dbgd[nm] = nc.declare_dram_parameter(nm, shp, F32, isOutput=True)

    cls_d = nc.declare_dram_parameter("cls", [NCLS, R], F32R, isOutput=True)
    rf_d = nc.declare_dram_parameter("rf", [RED, R], F32R, isOutput=True)

    dma = nc.sync.dma_start
    AF = mybir.ActivationFunctionType
    AL = mybir.AluOpType
    AX = mybir.AxisListType

    with nc.allow_low_precision(reason="f32r tiles are fp32-width"), \
         tile.TileContext(nc) as tc, ExitStack() as es:
        const = es.enter_context(tc.tile_pool(name="const", bufs=1))
        stream = es.enter_context(tc.tile_pool(name="stream", bufs=1))
        scr = es.enter_context(tc.tile_pool(name="scr", bufs=2))
        fcw = es.enter_context(tc.tile_pool(name="fcw", bufs=1))

        # ---- constants
        ident = const.tile([128, 128], F32R, name="ident")
        dma(out=ident, in_=ident_d[:, :])
        ones = const.tile([128, 1], F32R, name="ones")
        dma(out=ones, in_=ones_d[:, :])
        onesr = const.tile([1, 128], F32R, name="onesr")
        dma(out=onesr, in_=onesr_d[:, :])
        erep = const.tile([R, R * 128], F32R, name="erep")
        dma(out=erep, in_=e_d[:, :])
        e3 = const.tile([128, R * R], F32R, name="e3")
        dma(out=e3, in_=e3_d[:, :])
        gng = const.tile([128, DEPTH * MH], F32, name="gng")
        gnb = const.tile([128, DEPTH * MH], F32, name="gnb")
        for d in range(DEPTH):
            for m in range(MH):
                col = d * MH + m
                dma(out=gng[:, col:col + 1],
                    in_=gng_d[d, m * 128:(m + 1) * 128].rearrange("(p o) -> p o", o=1))
                dma(out=gnb[:, col:col + 1],
                    in_=gnb_d[d, m * 128:(m + 1) * 128].rearrange("(p o) -> p o", o=1))
        eps = const.tile([1, 1], F32, name="eps")
        nc.vector.memset(eps, 1e-5)
        neg1 = const.tile([128, 1], F32, name="neg1")
        nc.vector.memset(neg1, -1.0)

        wfc1 = [fcw.tile([128, HID], F32R, name=f"wfc1_{k}") for k in range(MR)]
        for k in range(MR):
            dma(out=wfc1[k], in_=wfc1T_d[k * 128:(k + 1) * 128, :])
        wfc2 = [fcw.tile([128, NCLS], F32R, name=f"wfc2_{k}") for k in range(MR)]
        for k in range(MR):
            dma(out=wfc2[k], in_=wfc2T_d[k * 128:(k + 1) * 128, :])
        roi_cT = [fcw.tile([128, R], F32R, name=f"roi{m}") for m in range(MR)]

        # x_hr and vp live from P5 to the end; open early for LIFO stacking
        hrx = es.enter_context(tc.tile_pool(name="hrx", bufs=1))
        x_hr = [hrx.tile([128, R, HP, WP], BF16, name=f"xhr{m}") for m in range(MH)]
        vp = [hrx.tile([128, R, HP, WP], BF16, name=f"vp{m}") for m in range(MH)]
        for m in range(MH):
            nc.vector.memset(x_hr[m][:, :, :, :], 0.0)
            nc.vector.memset(vp[m][:, :, :, :], 0.0)

        # ============ P1: temporal mean + conv_reduce -> featsT [pix, c]
        with tc.tile_pool(name="ftp", bufs=1) as ftp:
            featsT = [ftp.tile([128, RED], F32R, name=f"fT{m}") for m in range(3)]
            spT = [ftp.tile([128, NB], F32R, name=f"sp{m}") for m in range(3)]
            for m, (off, cnt) in enumerate(PIX_CH):
                dma(out=spT[m][:cnt, :], in_=spT_d[off:off + cnt, :])

            with tc.tile_pool(name="fcp", bufs=1) as fcp, \
                 tc.tile_pool(name="psA", bufs=1, space="PSUM") as psA:
                feats_cp = [fcp.tile([128, P], F32R, name=f"fcp{k}")
                            for k in range(KC)]
                psT = [psA.tile([128, RED], F32, name=f"psT{m}", tag=f"psT{m}")
                       for m in range(3)]
                for k in range(KC):
                    xt = fcp.tile([128, T * P], F32, name="xt", tag="xt", bufs=3)
                    dma(out=xt, in_=x_d[k * 128:(k + 1) * 128, :])
                    xv = _view(xt, 0, [(1, P), (P, T)])  # sum over t innermost
                    nc.vector.tensor_reduce(out=feats_cp[k], in_=xv,
                                            axis=AX.X, op=AL.add)
                    wr = stream.tile([128, RED], F32R, name="wr", tag="wr", bufs=3)
                    dma(out=wr, in_=wrT_d[k * 128:(k + 1) * 128, :])
                    for m, (off, cnt) in enumerate(PIX_CH):
                        for hh in range(2):
                            nc.tensor.matmul(
                                psT[m][:cnt, hh * 512:(hh + 1) * 512],
                                feats_cp[k][:, off:off + cnt],
                                wr[:, hh * 512:(hh + 1) * 512],
                                start=(k == 0), stop=(k == KC - 1))
                for m, (off, cnt) in enumerate(PIX_CH):
                    nc.scalar.copy(featsT[m][:cnt, :], psT[m][:cnt, :])

            # ============ P2+P3: transpose -> feats_red; roi bins -> roi_cT
            with tc.tile_pool(name="frp", bufs=1) as frp:
                feats_red = [frp.tile([128, P], F32R, name=f"fr{m}")
                             for m in range(MR)]
                with tc.tile_pool(name="psB", bufs=2, space="PSUM") as psB:
                    for m in range(MR):
                        for pc, (off, cnt) in enumerate(PIX_CH):
                            tp = psB.tile([128, 128], F32R, name="ptr", tag="ptr")
                            nc.tensor.transpose(
                                tp[:, :cnt], featsT[pc][:cnt, m * 128:(m + 1) * 128],
                                ident[:cnt, :cnt])
                            nc.scalar.copy(feats_red[m][:, off:off + cnt], tp[:, :cnt])
                    for m in range(MR):
                        pb = psB.tile([128, NB], F32, name="pbin", tag="pbin")
                        for pc, (off, cnt) in enumerate(PIX_CH):
                            nc.tensor.matmul(pb,
                                             featsT[pc][:cnt, m * 128:(m + 1) * 128],
                                             spT[pc][:cnt, :],
                                             start=(pc == 0), stop=(pc == 2))
                        pbv = _view(pb, 0, [(49, R), (1, 49)])
                        nc.vector.tensor_reduce(out=roi_cT[m], in_=pbv,
                                                axis=AX.X, op=AL.max)

                # ============ P4: conv1 bg (shared) + actor parts
                with tc.tile_pool(name="c2p", bufs=1) as c2p:
                    bg_sb = [c2p.tile([128, P], F32, name=f"bg{m}")
                             for m in range(MH)]
                    actor = [c2p.tile([128, R], F32, name=f"acr{m}")
                             for m in range(MH)]
                    with tc.tile_pool(name="psC", bufs=1, space="PSUM") as psC:
                        pbg = [psC.tile([128, P], F32, name=f"pbg{m}", tag=f"pbg{m}")
                               for m in range(MH)]
                        pac = [psC.tile([128, R], F32, name=f"pac{m}", tag=f"pac{m}")
                               for m in range(MH)]
                        for k in range(MR):
                            wb = stream.tile([128, HID], F32R, name="w1",
                                             tag="w1", bufs=3)
                            dma(out=wb, in_=w1bgT_d[k * 128:(k + 1) * 128, :])
                            for m in range(MH):
                                nc.tensor.matmul(pbg[m], wb[:, m * 128:(m + 1) * 128],
                                                 feats_red[k],
                                                 start=(k == 0), stop=(k == MR - 1))
                        for k in range(MR):
                            wa = stream.tile([128, HID], F32R, name="w1a",
                                             tag="w1", bufs=3)
                            dma(out=wa, in_=w1acT_d[k * 128:(k + 1) * 128, :])
                            for m in range(MH):
                                nc.tensor.matmul(pac[m], wa[:, m * 128:(m + 1) * 128],
                                                 roi_cT[k],
                                                 start=(k == 0), stop=(k == MR - 1))
                        for m in range(MH):
                            nc.scalar.copy(bg_sb[m], pbg[m])
                            nc.scalar.copy(actor[m], pac[m])
                        if dbg:
                            for m in range(MH):
                                dma(out=dbgd["dbg_bg"][m, :, :], in_=bg_sb[m])
                                dma(out=dbgd["dbg_ac"][m, :, :], in_=actor[m])

                    # ======== P5: conv2 (VALID) + relu + maxpool -> x_hr
                    mp = [c2p.tile([128, H, W], F32, name=f"mp{m}")
                          for m in range(MH)]
                    for m in range(MH):
                        nc.vector.memset(mp[m][:, :, :], 0.0)
                    with tc.tile_pool(name="w2p", bufs=1) as w2p, \
                         tc.tile_pool(name="psD", bufs=2, space="PSUM") as psD:
                        for oh in range(2):
                            # per cin-chunk tiles holding this oc-half's weights
                            w2 = [w2p.tile([128, 9, HID // 2], BF16, name=f"w2_{k}",
                                           tag=f"w2_{k}") for k in range(MH)]
                            for k in range(MH):
                                dma(out=w2[k],
                                    in_=w2T_d[:, k * 128:(k + 1) * 128,
                                              oh * 256:(oh + 1) * 256]
                                    .rearrange("s p c -> p s c"))
                            for r in range(R):
                                inter = [c2p.tile([128, P], BF16, name=f"int{m}",
                                                  tag=f"int{m}", bufs=2)
                                         for m in range(MH)]
                                for m in range(MH):
                                    nc.scalar.activation(
                                        out=inter[m], in_=bg_sb[m], func=AF.Relu,
                                        bias=actor[m][:, r:r + 1], scale=1.0)
                                if dbg and oh == 0 and r == 0:
                                    for m in range(MH):
                                        nc.gpsimd.dma_start(
                                            out=dbgd["dbg_int"][m, :, :],
                                            in_=inter[m][:, :])
                                for j in range(2):
                                    oc = oh * 2 + j
                                    pc2 = psD.tile([128, H2 * W2], F32,
                                                   name="pc2", tag="pc2")
                                    first = True
                                    for k in range(MH):
                                        for s in range(9):
                                            dy, dx = s // 3, s % 3
                                            rhs = _view(inter[k], dy * W + dx,
                                                        [(W, H2), (1, W2)])
                                            nc.tensor.matmul(
                                                pc2,
                                                w2[k][:, s, j * 128:(j + 1) * 128],
                                                rhs, start=first,
                                                stop=(k == MH - 1 and s == 8))
                                            first = False
                                    nc.scalar.activation(
                                        out=mp[oc][:, 1:1 + H2, 1:1 + W2],
                                        in_=pc2.rearrange("p (a b) -> p a b", b=W2),
                                        func=AF.Relu)
                                    win = _view(mp[oc], 0,
                                                [(2 * W, H3), (2, W3), (W, 3), (1, 3)])
                                    nc.vector.tensor_reduce(
                                        out=x_hr[oc][:, r, 1:1 + H3, 1:1 + W3],
                                        in_=win, axis=AX.XY, op=AL.max)

        # ============ P6: HR2O x DEPTH
        if dbg:
            for m in range(MH):
                nc.gpsimd.dma_start(out=dbgd["dbg_xhr0"][m, :, :],
                    in_=x_hr[m].rearrange("p a b c -> p (a b c)"))
        hrp = es.enter_context(tc.tile_pool(name="hrp", bufs=1))
        qsb = [hrp.tile([128, R, P3], F32, name=f"q{m}") for m in range(MH)]
        ksb = [hrp.tile([128, R, P3], F32, name=f"k{m}") for m in range(MH)]
        vsb = [hrp.tile([128, R, P3], F32, name=f"v{m}") for m in range(MH)]
        att_sm = hrp.tile([R, P3, R], F32R, name="attsm")
        st = [hrp.tile([128, 2 * R], F32R, name=f"st{m}") for m in range(MH)]
        mrb = hrp.tile([128, 2 * R], F32, name="mrb")
        al = [hrp.tile([128, R], F32, name=f"al{m}") for m in range(MH)]
        be = [hrp.tile([128, R], F32, name=f"be{m}") for m in range(MH)]

        def conv3x3_same(w_dram, src_tiles, sink):
            """3x3 SAME conv on [HID, 10, (7,10)] in padded (9,12) tiles.
            (kc, s)-outer streams each weight tile once; 8 psum accumulators."""
            with tc.tile_pool(name="pcv", bufs=1, space="PSUM") as pcvp:
                pcv = [pcvp.tile([128, 5 * P3], F32, name=f"pcv{i}", tag=f"pcv{i}")
                       for i in range(8)]
                for k in range(MH):
                    for s in range(9):
                        dy, dx = s // 3, s % 3
                        wt = stream.tile([128, HID], BF16, name="wc",
                                         tag="wc", bufs=8)
                        dma(out=wt, in_=w_dram[s, k * 128:(k + 1) * 128, :])
                        for oc in range(MH):
                            for g in range(2):
                                rhs = _view(src_tiles[k],
                                            g * 5 * PP + dy * WP + dx,
                                            [(PP, 5), (WP, H3), (1, W3)])
                                nc.tensor.matmul(
                                    pcv[oc * 2 + g],
                                    wt[:, oc * 128:(oc + 1) * 128], rhs,
                                    start=(k == 0 and s == 0),
                                    stop=(k == MH - 1 and s == 8))
                for oc in range(MH):
                    for g in range(2):
                        sink(oc, g, pcv[oc * 2 + g])

        def mk_copy(dst, scale=None):
            def _sink(oc, g, ps):
                o = dst[oc][:, g * 5:(g + 1) * 5, :]
                src = ps.rearrange("p (a b) -> p a b", b=P3)
                if scale is None:
                    nc.scalar.copy(o, src)
                else:
                    nc.scalar.mul(o, src, scale)
            return _sink

        for d in range(DEPTH):
            conv3x3_same(wqT_d[d], x_hr, mk_copy(qsb, INV_SQRT_HID))
            conv3x3_same(wkT_d[d], x_hr, mk_copy(ksb))
            conv3x3_same(wvT_d[d], x_hr, mk_copy(vsb))
            if dbg and d == 0:
                for m in range(MH):
                    dma(out=dbgd["dbg_q"][m, :, :], in_=qsb[m].rearrange("p a b -> p (a b)"))
                    dma(out=dbgd["dbg_k"][m, :, :], in_=ksb[m].rearrange("p a b -> p (a b)"))
                    dma(out=dbgd["dbg_v"][m, :, :], in_=vsb[m].rearrange("p a b -> p (a b)"))

            with tc.tile_pool(name="psE", bufs=2, space="PSUM") as psE:
                # att[i, j, p] = sum_c q[c,i,p] k[c,j,p], softmax over j
                for hf, (po, pn) in enumerate(HF_CH):
                    patt = psE.tile([R, 40 * R], F32, name="patt", tag="patt")
                    first = True
                    for i in range(R):
                        for c in range(MH):
                            q_b = qsb[c][:, i, po:po + pn] \
                                .broadcast_to([128, pn, R])
                            k_b = _view(ksb[c], po, [(1, pn), (P3, R)])
                            prod = scr.tile([128, 40 * R], F32R, name="prod",
                                            tag="prod", bufs=4)
                            nc.vector.tensor_tensor(
                                out=_view(prod, 0, [(R, pn), (1, R)]),
                                in0=q_b, in1=k_b, op=AL.mult)
                            nc.tensor.matmul(
                                patt[:, :pn * R], e3[:, i * R:(i + 1) * R],
                                prod[:, :pn * R], start=first,
                                stop=(i == R - 1 and c == MH - 1))
                            first = False
                    pattv = _view(patt, 0, [(R, pn), (1, R)])
                    mx = scr.tile([R, 40], F32, name="mx", tag="mx")
                    nc.vector.tensor_reduce(out=mx[:, :pn], in_=pattv,
                                            axis=AX.X, op=AL.max)
                    att_c = scr.tile([R, 40 * R], F32, name="attc", tag="attc")
                    nc.vector.tensor_tensor(
                        out=_view(att_c, 0, [(R, pn), (1, R)]),
                        in0=pattv, in1=mx[:, :pn].broadcast_to([R, pn, R]),
                        op=AL.subtract)
                    att_e = scr.tile([R, 40 * R], F32, name="atte", tag="atte")
                    nc.scalar.activation(out=att_e[:, :pn * R],
                                         in_=att_c[:, :pn * R], func=AF.Exp)
                    sm = scr.tile([R, 40], F32, name="sm", tag="sm")
                    nc.vector.tensor_reduce(
                        out=sm[:, :pn], in_=_view(att_e, 0, [(R, pn), (1, R)]),
                        axis=AX.X, op=AL.add)
                    rs = scr.tile([R, 40], F32, name="rs", tag="rs")
                    nc.vector.reciprocal(out=rs[:, :pn], in_=sm[:, :pn])
                    nc.vector.tensor_tensor(
                        out=att_sm[:, po:po + pn, :],
                        in0=_view(att_e, 0, [(R, pn), (1, R)]),
                        in1=rs[:, :pn].broadcast_to([R, pn, R]), op=AL.mult)

                # attb (replicated) + virt accumulated straight into vp interior
                for hf, (po, pn) in enumerate(HF_CH):
                    attb = hrp.tile([128, R, 40, R], F32, name="attb",
                                    tag="attb", bufs=2)
                    for i in range(R):
                        prep = psE.tile([128, 40 * R], F32, name="prep",
                                        tag="prep")
                        nc.tensor.matmul(prep[:, :pn * R],
                                         erep[:, i * 128:(i + 1) * 128],
                                         att_sm[:, po:po + pn, :],
                                         start=True, stop=True)
                        nc.scalar.copy(attb[:, i, :pn, :],
                                       _view(prep, 0, [(R, pn), (1, R)]))
                    yo, yn = (1, 4) if hf == 0 else (5, 3)
                    for c in range(MH):
                        for i in range(R):
                            v_pj = _view(vsb[c], po, [(1, pn), (P3, R)])
                            p2 = scr.tile([128, 40 * R], F32, name="p2",
                                          tag="p2")
                            nc.vector.tensor_tensor(
                                out=_view(p2, 0, [(R, pn), (1, R)]),
                                in0=v_pj, in1=attb[:, i, :pn, :], op=AL.mult)
                            nc.vector.tensor_reduce(
                                out=vp[c][:, i, yo:yo + yn, 1:1 + W3],
                                in_=_view(p2, 0, [(10 * R, yn), (R, W3), (1, R)]),
                                axis=AX.X, op=AL.add)

                if dbg and d == 0:
                    dma(out=dbgd["dbg_att"][:, :],
                        in_=att_sm.rearrange("p a b -> p (a b)").bitcast(F32))
                    for m in range(MH):
                        nc.gpsimd.dma_start(out=dbgd["dbg_vpraw"][m, :, :],
                            in_=vp[m].rearrange("p a b c -> p (a b c)"))

                # GroupNorm(1, HID) per actor over (c, p): stats from vp interior
                for c in range(MH):
                    vin = vp[c][:, :, 1:1 + H3, 1:1 + W3]
                    nc.vector.tensor_reduce(out=st[c][:, 0:R], in_=vin,
                                            axis=AX.XY, op=AL.add)
                    sq = scr.tile([128, R * P3], F32, name="sq", tag="sq")
                    nc.vector.tensor_tensor(
                        out=sq.rearrange("p (a b c) -> p a b c", b=H3, c=W3),
                        in0=vin, in1=vin, op=AL.mult)
                    nc.vector.tensor_reduce(
                        out=st[c][:, R:2 * R],
                        in_=sq.rearrange("p (a b) -> p a b", b=P3),
                        axis=AX.X, op=AL.add)
                pgn = psE.tile([1, 2 * R], F32, name="pgn", tag="pgn")
                for c in range(MH):
                    nc.tensor.matmul(pgn, ones, st[c], start=(c == 0),
                                     stop=(c == MH - 1))
                mr = scr.tile([1, 2 * R], F32R, name="mr", tag="mr")
                ninv = 1.0 / float(HID * P3)
                nc.vector.tensor_scalar(out=mr, in0=pgn, scalar1=ninv,
                                        scalar2=None, op0=AL.mult)
                musq = scr.tile([1, R], F32, name="musq", tag="musq")
                nc.vector.tensor_tensor(out=musq, in0=mr[:, 0:R].bitcast(F32),
                                        in1=mr[:, 0:R].bitcast(F32), op=AL.mult)
                var = scr.tile([1, R], F32, name="var", tag="var")
                nc.vector.tensor_tensor(out=var, in0=mr[:, R:2 * R].bitcast(F32),
                                        in1=musq, op=AL.subtract)
                sd = scr.tile([1, R], F32, name="sd", tag="sd")
                nc.scalar.activation(out=sd, in_=var, func=AF.Sqrt,
                                     bias=eps[:1, :], scale=1.0)
                nc.vector.reciprocal(out=mr[:, R:2 * R], in_=sd)
                pgr = psE.tile([128, 2 * R], F32, name="pgr", tag="pgr")
                nc.tensor.matmul(pgr, onesr, mr, start=True, stop=True)
                nc.scalar.copy(mrb, pgr)

            for c in range(MH):
                gi = d * MH + c
                nc.vector.tensor_scalar_mul(out=al[c], in0=mrb[:, R:2 * R],
                                            scalar1=gng[:, gi:gi + 1])
                ma = scr.tile([128, R], F32, name="ma", tag="ma")
                nc.vector.tensor_tensor(out=ma, in0=mrb[:, 0:R], in1=al[c],
                                        op=AL.mult)
                nc.vector.tensor_scalar_mul(out=ma, in0=ma, scalar1=neg1)
                nc.vector.tensor_scalar_add(out=be[c], in0=ma,
                                            scalar1=gnb[:, gi:gi + 1])
                for i in range(R):
                    vint = vp[c][:, i, 1:1 + H3, 1:1 + W3]
                    nc.scalar.activation(out=vint, in_=vint, func=AF.Relu,
                                         scale=al[c][:, i:i + 1],
                                         bias=be[c][:, i:i + 1])

            if dbg and d == 0:
                for m in range(MH):
                    nc.gpsimd.dma_start(out=dbgd["dbg_vpact"][m, :, :],
                        in_=vp[m].rearrange("p a b c -> p (a b c)"))

            # o-conv + residual add into x_hr (in place)
            def mk_res(oc, g, ps):
                tgt = x_hr[oc][:, g * 5:(g + 1) * 5, 1:1 + H3, 1:1 + W3]
                nc.vector.tensor_tensor(
                    out=tgt,
                    in0=ps.rearrange("p (a b c) -> p a b c", b=H3, c=W3),
                    in1=tgt, op=AL.add)
            conv3x3_same(woT_d[d], vp, mk_res)
            if dbg and d == 0:
                for m in range(MH):
                    nc.gpsimd.dma_start(out=dbgd["dbg_xhr1"][m, :, :],
                        in_=x_hr[m].rearrange("p a b c -> p (a b c)"))

        # ============ P7: hof, fc1, fc2, outputs
        with tc.tile_pool(name="psF", bufs=1, space="PSUM") as psF:
            hof = [hrp.tile([128, R], F32R, name=f"hof{m}") for m in range(MH)]
            for c in range(MH):
                nc.vector.tensor_reduce(
                    out=hof[c], in_=x_hr[c][:, :, 1:1 + H3, 1:1 + W3],
                    axis=AX.XY, op=AL.add)
            ofc1 = [hrp.tile([128, R], F32R, name=f"ofc{m}") for m in range(MH)]
            pfc = [psF.tile([128, R], F32, name=f"pfc{m}", tag=f"pfc{m}")
                   for m in range(MH)]
            for k in range(MR):
                for m in range(MH):
                    nc.tensor.matmul(pfc[m], wfc1[k][:, m * 128:(m + 1) * 128],
                                     roi_cT[k], start=(k == 0), stop=(k == MR - 1))
            for m in range(MH):
                nc.scalar.activation(out=ofc1[m], in_=pfc[m], func=AF.Relu)
            if dbg:
                for m in range(MH):
                    dma(out=dbgd["dbg_hof"][m, :, :], in_=hof[m][:, :].bitcast(F32))
                    dma(out=dbgd["dbg_ofc"][m, :, :], in_=ofc1[m][:, :].bitcast(F32))
            pcls = psF.tile([NCLS, R], F32, name="pcls", tag="pcls")
            for k in range(MR):
                rhs = ofc1[k] if k < MH else hof[k - MH]
                nc.tensor.matmul(pcls, wfc2[k], rhs, start=(k == 0),
                                 stop=(k == MR - 1))
            cls_sb = fcw.tile([NCLS, R], F32R, name="cls_sb")
            nc.scalar.copy(cls_sb, pcls)
            dma(out=cls_d[:, :], in_=cls_sb)
            for m in range(MR):
                dma(out=rf_d[m * 128:(m + 1) * 128, :], in_=roi_cT[m])

    if split:
        _split_waits(nc)
    return nc


# ---------------------------------------------------------------- host side
def _roi_matrix(rois_px):
    """rois_px [R,5] pixel coords -> S^T [P, R*49] f32 (roi_align + bin mean),
    float32 coordinate arithmetic matching the jax reference."""
    S = np.zeros((NB, P), np.float64)
    grid = (np.arange(ROI_SP, dtype=np.float32)[:, None]
            + (np.arange(SR, dtype=np.float32)[None, :] + 0.5)
            / np.float32(SR)).reshape(-1)
    for r in range(R):
        x1, y1, x2, y2 = [np.float32(v) for v in rois_px[r, 1:5]]
        bw = np.maximum(x2 - x1, np.float32(1.0)) / np.float32(ROI_SP)
        bh = np.maximum(y2 - y1, np.float32(1.0)) / np.float32(ROI_SP)
        ys = (y1 + grid * bh).astype(np.float32)
        xs = (x1 + grid * bw).astype(np.float32)
        ym = (ys >= -1.0) & (ys <= H)
        xm = (xs >= -1.0) & (xs <= W)
        yv = np.clip(ys, np.float32(0.0), np.float32(H - 1))
        xv = np.clip(xs, np.float32(0.0), np.float32(W - 1))
        y0 = np.floor(yv).astype(np.int64)
        x0 = np.floor(xv).astype(np.int64)
        yh = np.minimum(y0 + 1, H - 1)
        xh = np.minimum(x0 + 1, W - 1)
        ly = (yv - y0).astype(np.float64)
        lx = (xv - x0).astype(np.float64)
        wgt = 1.0 / (SR * SR)
        for iy in range(ROI_SP * SR):
            if not ym[iy]:
                continue
            for ix in range(ROI_SP * SR):
                if not xm[ix]:
                    continue
                b = r * 49 + (iy // SR) * ROI_SP + (ix // SR)
                S[b, y0[iy] * W + x0[ix]] += wgt * (1 - ly[iy]) * (1 - lx[ix])
                S[b, y0[iy] * W + xh[ix]] += wgt * (1 - ly[iy]) * lx[ix]
                S[b, yh[iy] * W + x0[ix]] += wgt * ly[iy] * (1 - lx[ix])
                S[b, yh[iy] * W + xh[ix]] += wgt * ly[iy] * lx[ix]
    return np.ascontiguousarray(S.T.astype(np.float32))


_NC_CACHE = {}


def _get_nc():
    if "nc" not in _NC_CACHE:
        _NC_CACHE["nc"] = build_nc()
    return _NC_CACHE["nc"]


def _conv_T(w):  # [O, I, 3, 3] -> [9, I, O] in bf16
    return np.ascontiguousarray(
        w.transpose(2, 3, 1, 0).reshape(9, w.shape[1], w.shape[0])
        .astype(ml_dtypes.bfloat16))


def kernel(x, rois, w_reduce, w_conv1, w_conv2, w_q, w_k, w_v, w_o,
           gn_g, gn_b, w_fc1, w_fc2):
    f32 = np.float32
    x = np.asarray(x, f32)
    rois = np.asarray(rois, f32)
    nc = _get_nc()

    wrT = np.ascontiguousarray((np.asarray(w_reduce, f32) * (1.0 / T)).T)
    w1 = np.asarray(w_conv1, f32)
    w1bgT = np.ascontiguousarray(w1[:, :RED].T)
    w1acT = np.ascontiguousarray(w1[:, RED:].T)
    w2T = _conv_T(np.asarray(w_conv2, f32))
    wqT = np.stack([_conv_T(np.asarray(w_q, f32)[d]) for d in range(DEPTH)])
    wkT = np.stack([_conv_T(np.asarray(w_k, f32)[d]) for d in range(DEPTH)])
    wvT = np.stack([_conv_T(np.asarray(w_v, f32)[d]) for d in range(DEPTH)])
    woT = np.stack([_conv_T(np.asarray(w_o, f32)[d]) for d in range(DEPTH)])
    wfc1T = np.ascontiguousarray(np.asarray(w_fc1, f32).T)
    wfc2T = np.ascontiguousarray(np.asarray(w_fc2, f32).T)
    wfc2T[HID:, :] *= 1.0 / P3  # fold hof spatial mean
    ident = np.eye(128, dtype=f32)
    ones = np.ones((128, 1), f32)
    onesr = np.ones((1, 128), f32)
    erep = np.zeros((R, R * 128), f32)
    for i in range(R):
        erep[i, i * 128:(i + 1) * 128] = 1.0
    e3 = np.zeros((128, R * R), f32)
    for i in range(R):
        e3[:, i * R + i] = 1.0

    scale = np.array([1.0, W, H, W, H], f32)
    in_maps = []
    for c in range(N_CORES):
        rois_px = rois[c * R:(c + 1) * R] * scale[None, :]
        in_maps.append({
            "x": np.ascontiguousarray(x[c].reshape(CIN, T * P)),
            "wrT": wrT, "spT": _roi_matrix(rois_px),
            "w1bgT": w1bgT, "w1acT": w1acT, "w2T": w2T,
            "wqT": wqT, "wkT": wkT, "wvT": wvT, "woT": woT,
            "gng": np.asarray(gn_g, f32), "gnb": np.asarray(gn_b, f32),
            "wfc1T": wfc1T, "wfc2T": wfc2T,
            "ident": ident, "ones": ones, "onesr": onesr,
            "erep": erep, "e3": e3,
        })

    res = run_bass_kernel_spmd(nc, in_maps, core_ids=list(range(N_CORES)),
                               trace=False)
    _NC_CACHE["last_res"] = res
    cls = np.empty((B * R, NCLS), f32)
    rf = np.empty((B * R, RED), f32)
    for c in range(N_CORES):
        cls[c * R:(c + 1) * R] = res.results[c]["cls"].T
        rf[c * R:(c + 1) * R] = res.results[c]["rf"].T
    return cls, rf
